# revision 41
# baseline (speedup 1.0000x reference)
"""Contrastive loss (SimCLR-style) on 8 TRN2 NeuronCores.

loss = -mean(diag(log_softmax(zi_n @ zj_n^T / T)))  with zi_n, zj_n L2-normalized,
N=4096, D=256, T=0.5.

Sharding: 16 quarter-block cover. Core pair (2q, 2q+1) owns zi quarter q
(1024 rows). Core 2q gets zj cols = quarters {q, q+1}; core 2q+1 gets
{q+2, q+3} (mod 4). Every (row-quarter, col-quarter) block is computed
exactly once; every core's zjA input aligns row-for-row with its zi input,
so the diagonal block is always in zjA for even cores. 3MB HBM per core.

Per core: l2-normalize zj (bf16, quake rsqrt on DVE), zi norms on the
otherwise-idle ScalarE (Square+accum shares the exp table set), xbar
transposes split across both HWDGE queues, bf16 matmul with PE pre-warm
against the HAM clock gate, fused exp+row-sum on ScalarE with
per-partition scale 2/||zi||. Device outputs per-row partial exp-sums +
diag logits; host adds core-pair partials, takes ln, subtracts diag, means.
"""

import numpy as np

import concourse.bass as bass
import concourse.bacc as bacc
import concourse.tile as tile
import concourse.bass_utils as bass_utils
from concourse import mybir
from concourse.tile_rust import add_dep_helper

N = 4096
D = 256
NCORES = 8
NQ = N // 4          # 1024 rows per quarter (zi rows per core)
P = 128
CH = NQ // P         # 8 row-chunks (rows per partition)
HC = CH // 2
KH = D // P          # 2 contraction halves
MAGIC = 0x5F3759DF
NWARM = 16           # dummy matmuls to lift the PE HAM clock gate

F32 = mybir.dt.float32
U32 = mybir.dt.uint32
BF16 = mybir.dt.bfloat16
AF = mybir.ActivationFunctionType
ALU = mybir.AluOpType
AX = mybir.AxisListType


def build_nc():
    nc = bacc.Bacc(
        "TRN2",
        target_bir_lowering=False,
        debug=False,
        enable_asserts=False,
    )
    z_i = nc.dram_tensor("z_i", (NQ, D), F32, kind="ExternalInput").ap()
    z_ja = nc.dram_tensor("z_ja", (NQ, D), F32, kind="ExternalInput").ap()
    z_jb = nc.dram_tensor("z_jb", (NQ, D), F32, kind="ExternalInput").ap()
    out = nc.dram_tensor("out", (P, 16), F32, kind="ExternalOutput").ap()

    with tile.TileContext(nc) as tc:
        with (
            tc.tile_pool(name="const", bufs=1) as const,
            tc.tile_pool(name="big", bufs=1) as big,
            tc.tile_pool(name="work", bufs=2) as work,
            tc.tile_pool(name="stat", bufs=1) as stat,
            tc.tile_pool(name="psum", bufs=1, space="PSUM") as psum,
            tc.tile_pool(name="warm", bufs=1, space="PSUM") as warmp,
        ):
            # pin the exp ACT table set at t=0 (also covers square/copy)
            dummy = const.tile([1, 1], F32)
            nc.vector.memset(dummy, 1.0)
            nc.scalar.activation(out=dummy, in_=dummy, func=AF.Exp)

            magic = const.tile([P, 16], U32)
            nc.vector.memset(magic, MAGIC)

            # ---- loads (sync queue, FIFO): zjA halves -> zi halves -> zjB
            def load_halves(dst, src):
                for hh in range(2):
                    nc.sync.dma_start(
                        out=dst[:, hh * HC : (hh + 1) * HC, :],
                        in_=src[hh * 512 : (hh + 1) * 512, :].rearrange(
                            "(p c) d -> p c d", p=P
                        ),
                    )

            zja_f = big.tile([P, CH, D], F32)
            load_halves(zja_f, z_ja)
            zi_f = big.tile([P, CH, D], F32)
            load_halves(zi_f, z_i)
            zjb_f = big.tile([P, CH, D], F32)
            nc.sync.dma_start(
                out=zjb_f, in_=z_jb.rearrange("(p c) d -> p c d", p=P)
            )

            # ---- PE warm-up against the HAM clock gate
            wsrc = const.tile([P, 512], BF16)
            nc.vector.memset(wsrc, 0.0)
            wp = warmp.tile([P, 512], F32, tag="wp")
            for _ in range(NWARM):
                nc.tensor.matmul(
                    wp, lhsT=wsrc[:, :P], rhs=wsrc, start=True, stop=True,
                    skip_group_check=True,
                )

            dve_tail = [None]

            def chain(bi):
                if dve_tail[0] is not None:
                    add_dep_helper(bi.ins, dve_tail[0], sync=False,
                                   reason="dve order")
                dve_tail[0] = bi.ins
                return bi

            def rsqrt_dve(a, y, w):
                """y[:,:w] = 1/sqrt(a[:,:w]): quake seed + 1 Newton step."""
                au = a.bitcast(U32)
                yu = y.bitcast(U32)
                sh = work.tile([P, 16], U32, tag="rsq_sh")
                chain(nc.vector.tensor_scalar(
                    out=sh[:, :w], in0=au, scalar1=1, scalar2=None,
                    op0=ALU.logical_shift_right,
                ))
                chain(nc.vector.tensor_sub(
                    out=yu, in0=magic[:, :w], in1=sh[:, :w]
                ))
                t1 = work.tile([P, 16], F32, tag="rsq_t1")
                chain(nc.vector.tensor_mul(out=t1[:, :w], in0=y, in1=y))
                chain(nc.vector.tensor_mul(out=t1[:, :w], in0=t1[:, :w], in1=a))
                chain(nc.vector.tensor_scalar(
                    out=t1[:, :w], in0=t1[:, :w], scalar1=-0.5, scalar2=1.5,
                    op0=ALU.mult, op1=ALU.add,
                ))
                chain(nc.vector.tensor_mul(out=y, in0=y, in1=t1[:, :w]))

            def norms_dve(zf, nrm):
                for c in range(CH):
                    sq = work.tile([P, D], BF16, tag="sq")
                    chain(nc.vector.scalar_tensor_tensor(
                        out=sq, in0=zf[:, c, :], scalar=1.0, in1=zf[:, c, :],
                        op0=ALU.mult, op1=ALU.mult,
                        accum_out=nrm[:, c : c + 1],
                    ))

            def scale_dve(zf, rsq, zs):
                for c in range(CH):
                    chain(nc.vector.tensor_scalar_mul(
                        out=zs[:, c, :], in0=zf[:, c, :],
                        scalar1=rsq[:, c : c + 1],
                    ))

            def xpose_halves(src_bf, dstT):
                """Transpose [P, CH, D] bf16 in 2 halves on both HWDGE queues."""
                flat = src_bf.rearrange("p c d -> p (c d)")
                nc.sync.dma_start_transpose(
                    out=dstT[:, :CH, :], in_=flat[:, : HC * D]
                )
                nc.sync.dma_start_transpose(
                    out=dstT[:, CH:, :], in_=flat[:, HC * D :]
                )

            # ---- DVE: norms-A -> rsq-A -> scale-A (shortest path to zjaT);
            # cast-i after (zi-load-gated anyway); zjaT ahead of ziT in FIFO
            nrm_a = stat.tile([P, CH], F32)
            norms_dve(zja_f, nrm_a)
            rsq_a = stat.tile([P, CH], F32)
            rsqrt_dve(nrm_a, rsq_a, CH)
            zjas = big.tile([P, CH, D], BF16)
            scale_dve(zja_f, rsq_a, zjas)
            zjaT = big.tile([P, CH * KH, P], BF16)
            xpose_halves(zjas, zjaT)
            zjaT_r = zjaT.rearrange("do (c h) m -> do c h m", h=KH)
            zi_bf = big.tile([P, CH, D], BF16)
            chain(nc.vector.tensor_copy(out=zi_bf, in_=zi_f))
            ziT = big.tile([P, CH * KH, P], BF16)
            xpose_halves(zi_bf, ziT)
            ziT_r = ziT.rearrange("do (c h) m -> do c h m", h=KH)

            # ---- zi norms on ScalarE (idle window; square shares exp tables)
            nrm_i = stat.tile([P, CH], F32)
            for c in range(CH):
                sqa = work.tile([P, D], BF16, tag="sqa")
                nc.scalar.activation(
                    out=sqa, in_=zi_f[:, c, :], func=AF.Square,
                    accum_out=nrm_i[:, c : c + 1],
                )
            rsq_i = stat.tile([P, CH], F32)
            rsqrt_dve(nrm_i, rsq_i, CH)
            s2 = stat.tile([P, CH], F32)
            chain(nc.vector.tensor_scalar(
                out=s2, in0=rsq_i, scalar1=2.0, scalar2=None, op0=ALU.mult
            ))
            asch = stat.tile([P, CH], F32)
            chain(nc.vector.tensor_scalar(
                out=asch, in0=s2, scalar1=12102203.161561485, scalar2=None,
                op0=ALU.mult,
            ))
            rsd = stat.tile([P, CH], F32)
            nc.vector.memset(rsd, 0.0)

            # ---- main compute tiles
            rs = psum.tile([P, 2 * CH], F32, tag="rs")  # 0-7: A, 8-15: B
            nc.vector.memset(rs[:, CH:], 0.0)  # DVE-handled B-chunks stay 0

            def half_tile(c, zT_r, g, on_dve=False):
                pt = psum.tile([P, 1024], F32, tag=f"pt{(g * CH + c) % 3}")
                for h in range(KH):
                    for jj in range(2):
                        nc.tensor.matmul(
                            pt[:, jj * 512 : (jj + 1) * 512],
                            lhsT=ziT_r[:, c, h, :],
                            rhs=zT_r[:, jj * 4 : jj * 4 + 4, h, :],
                            start=(h == 0),
                            stop=(h == KH - 1),
                        )
                if not on_dve:
                    nc.scalar.activation(
                        out=pt, in_=pt, func=AF.Exp,
                        scale=s2[:, c : c + 1],
                        accum_out=rs[:, g * CH + c : g * CH + c + 1],
                    )
                else:
                    # Schraudolph exp on DVE: bitcast(i32(A*s2*x + B)) ~ exp;
                    # only used late, when DVE is past all preprocessing
                    ub = work.tile([P, 1024], U32, tag="sch")
                    nc.vector.tensor_scalar(
                        out=ub, in0=pt,
                        scalar1=asch[:, c : c + 1], scalar2=1064986823.0,
                        op0=ALU.mult, op1=ALU.add,
                    )
                    jb = work.tile([P, 1024], BF16, tag="schj")
                    nc.vector.tensor_scalar(
                        out=jb, in0=ub.bitcast(F32),
                        scalar1=1.0, scalar2=0.0, op0=ALU.mult, op1=ALU.add,
                        accum_out=rsd[:, c : c + 1],
                    )

            for c in range(CH):
                half_tile(c, zjaT_r, 0)

            # ---- zjB chain (overlaps A-phase compute)
            nrm_b = stat.tile([P, CH], F32)
            norms_dve(zjb_f, nrm_b)
            rsq_b = stat.tile([P, CH], F32)
            rsqrt_dve(nrm_b, rsq_b, CH)
            zjbs = big.tile([P, CH, D], BF16)
            scale_dve(zjb_f, rsq_b, zjbs)
            zjbT = big.tile([P, CH * KH, P], BF16)
            xpose_halves(zjbs, zjbT)
            zjbT_r = zjbT.rearrange("do (c h) m -> do c h m", h=KH)

            # ---- diag (even cores use it): zi_bf . zjas, then * s2
            dt = stat.tile([P, CH], F32)
            for c in range(CH):
                sqw = work.tile([P, D], BF16, tag="sq")
                chain(nc.vector.scalar_tensor_tensor(
                    out=sqw, in0=zi_bf[:, c, :], scalar=1.0,
                    in1=zjas[:, c, :],
                    op0=ALU.mult, op1=ALU.mult,
                    accum_out=dt[:, c : c + 1],
                ))
            dg = stat.tile([P, CH], F32)
            chain(nc.vector.tensor_mul(out=dg, in0=dt, in1=s2))

            for c in range(CH):
                half_tile(c, zjbT_r, 1, on_dve=(c in (1, 2, 4, 6)))

            # ---- output: [rsA+rsB | dg]; copies float early (deps only),
            # final DMA rides the idle sync queue so Act never issues it
            osb = stat.tile([P, 16], F32)
            nc.vector.tensor_copy(out=osb[:, CH:], in_=dg)
            nc.vector.tensor_copy(out=osb[:, :CH], in_=rs[:, :CH])
            nc.vector.tensor_add(
                out=osb[:, :CH], in0=osb[:, :CH], in1=rs[:, CH:]
            )
            nc.vector.tensor_add(
                out=osb[:, :CH], in0=osb[:, :CH], in1=rsd
            )
            nc.sync.dma_start(out=out, in_=osb)

    nc.compile()
    return nc


_NC = None


def _get_nc():
    global _NC
    if _NC is None:
        _NC = build_nc()
    return _NC


def make_in_maps(z_i, z_j):
    """Per-core inputs for the quarter-block cover."""
    Q = [np.ascontiguousarray(z_j[q * NQ : (q + 1) * NQ], np.float32)
         for q in range(4)]
    in_maps = []
    for q in range(4):
        zi_q = np.ascontiguousarray(z_i[q * NQ : (q + 1) * NQ], np.float32)
        in_maps.append(
            {"z_i": zi_q, "z_ja": Q[q], "z_jb": Q[(q + 1) % 4]}
        )
        in_maps.append(
            {"z_i": zi_q, "z_ja": Q[(q + 2) % 4], "z_jb": Q[(q + 3) % 4]}
        )
    return in_maps


def combine(results):
    """Host: add core-pair exp-sums, ln, subtract diag, mean.

    Row order inside rs/dg is a fixed permutation of the quarter's rows;
    the mean is permutation-invariant and the two cores of a pair share
    the same zi packing, so elementwise add + sum is exact.
    """
    total = 0.0
    for q in range(4):
        oa = results[2 * q]["out"].astype(np.float64)
        ob = results[2 * q + 1]["out"].astype(np.float64)
        rs_total = oa[:, :CH] + ob[:, :CH]   # [128, 8]
        dgq = oa[:, CH:16]                   # diag logits (even core's zjA)
        total += float(np.sum(np.log(rs_total) - dgq))
    return np.float32(total / N)


def kernel(z_i: np.ndarray, z_j: np.ndarray, **_unused) -> np.ndarray:
    z_i = np.ascontiguousarray(z_i, dtype=np.float32)
    z_j = np.ascontiguousarray(z_j, dtype=np.float32)
    nc = _get_nc()
    res = bass_utils.run_bass_kernel_spmd(
        nc, make_in_maps(z_i, z_j), core_ids=list(range(NCORES))
    )
    return combine(res.results)


# revision 42
# speedup vs baseline: 1.0462x; 1.0462x over previous
"""Contrastive loss (SimCLR-style) on 8 TRN2 NeuronCores.

loss = -mean(diag(log_softmax(zi_n @ zj_n^T / T)))  with zi_n, zj_n L2-normalized,
N=4096, D=256, T=0.5.

Sharding: 16 quarter-block cover. Core pair (2q, 2q+1) owns zi quarter q
(1024 rows). Core 2q gets zj cols = quarters {q, q+1}; core 2q+1 gets
{q+2, q+3} (mod 4). Every (row-quarter, col-quarter) block is computed
exactly once; every core's zjA input aligns row-for-row with its zi input,
so the diagonal block is always in zjA for even cores. 3MB HBM per core.

Per core: l2-normalize zj (bf16, quake rsqrt on DVE), zi norms on the
otherwise-idle ScalarE (Square+accum shares the exp table set), xbar
transposes split across both HWDGE queues, bf16 matmul with PE pre-warm
against the HAM clock gate, fused exp+row-sum on ScalarE with
per-partition scale 2/||zi||. Device outputs per-row partial exp-sums +
diag logits; host adds core-pair partials, takes ln, subtracts diag, means.
"""

import numpy as np

import concourse.bass as bass
import concourse.bacc as bacc
import concourse.tile as tile
import concourse.bass_utils as bass_utils
from concourse import mybir
from concourse.tile_rust import add_dep_helper

N = 4096
D = 256
NCORES = 8
NQ = N // 4          # 1024 rows per quarter (zi rows per core)
P = 128
CH = NQ // P         # 8 row-chunks (rows per partition)
HC = CH // 2
KH = D // P          # 2 contraction halves
MAGIC = 0x5F3759DF
NWARM = 16           # dummy matmuls to lift the PE HAM clock gate

F32 = mybir.dt.float32
U32 = mybir.dt.uint32
BF16 = mybir.dt.bfloat16
AF = mybir.ActivationFunctionType
ALU = mybir.AluOpType
AX = mybir.AxisListType


def build_nc():
    nc = bacc.Bacc(
        "TRN2",
        target_bir_lowering=False,
        debug=False,
        enable_asserts=False,
    )
    z_i = nc.dram_tensor("z_i", (NQ, D), F32, kind="ExternalInput").ap()
    z_ja = nc.dram_tensor("z_ja", (NQ, D), F32, kind="ExternalInput").ap()
    z_jb = nc.dram_tensor("z_jb", (NQ, D), F32, kind="ExternalInput").ap()
    out = nc.dram_tensor("out", (P, 16), F32, kind="ExternalOutput").ap()

    with tile.TileContext(nc) as tc:
        with (
            tc.tile_pool(name="const", bufs=1) as const,
            tc.tile_pool(name="big", bufs=1) as big,
            tc.tile_pool(name="work", bufs=2) as work,
            tc.tile_pool(name="stat", bufs=1) as stat,
            tc.tile_pool(name="psum", bufs=1, space="PSUM") as psum,
            tc.tile_pool(name="warm", bufs=1, space="PSUM") as warmp,
        ):
            # pin the exp ACT table set at t=0 (also covers square/copy)
            dummy = const.tile([1, 1], F32)
            nc.vector.memset(dummy, 1.0)
            nc.scalar.activation(out=dummy, in_=dummy, func=AF.Exp)

            magic = const.tile([P, 16], U32)
            nc.vector.memset(magic, MAGIC)

            # ---- loads (sync queue, FIFO): zjA halves -> zi halves -> zjB
            def load_halves(dst, src):
                for hh in range(2):
                    nc.sync.dma_start(
                        out=dst[:, hh * HC : (hh + 1) * HC, :],
                        in_=src[hh * 512 : (hh + 1) * 512, :].rearrange(
                            "(p c) d -> p c d", p=P
                        ),
                    )

            zja_f = big.tile([P, CH, D], F32)
            load_halves(zja_f, z_ja)
            zi_f = big.tile([P, CH, D], F32)
            load_halves(zi_f, z_i)
            zjb_f = big.tile([P, CH, D], F32)
            nc.sync.dma_start(
                out=zjb_f, in_=z_jb.rearrange("(p c) d -> p c d", p=P)
            )

            # ---- PE warm-up against the HAM clock gate
            wsrc = const.tile([P, 512], BF16)
            nc.vector.memset(wsrc, 0.0)
            wp = warmp.tile([P, 512], F32, tag="wp")
            for _ in range(NWARM):
                nc.tensor.matmul(
                    wp, lhsT=wsrc[:, :P], rhs=wsrc, start=True, stop=True,
                    skip_group_check=True,
                )

            dve_tail = [None]

            def chain(bi):
                if dve_tail[0] is not None:
                    add_dep_helper(bi.ins, dve_tail[0], sync=False,
                                   reason="dve order")
                dve_tail[0] = bi.ins
                return bi

            def rsqrt_dve(a, y, w):
                """y[:,:w] = 1/sqrt(a[:,:w]): quake seed + 1 Newton step."""
                au = a.bitcast(U32)
                yu = y.bitcast(U32)
                sh = work.tile([P, 16], U32, tag="rsq_sh")
                chain(nc.vector.tensor_scalar(
                    out=sh[:, :w], in0=au, scalar1=1, scalar2=None,
                    op0=ALU.logical_shift_right,
                ))
                chain(nc.vector.tensor_sub(
                    out=yu, in0=magic[:, :w], in1=sh[:, :w]
                ))
                t1 = work.tile([P, 16], F32, tag="rsq_t1")
                chain(nc.vector.tensor_mul(out=t1[:, :w], in0=y, in1=y))
                chain(nc.vector.tensor_mul(out=t1[:, :w], in0=t1[:, :w], in1=a))
                chain(nc.vector.tensor_scalar(
                    out=t1[:, :w], in0=t1[:, :w], scalar1=-0.5, scalar2=1.5,
                    op0=ALU.mult, op1=ALU.add,
                ))
                chain(nc.vector.tensor_mul(out=y, in0=y, in1=t1[:, :w]))

            def norms_dve(zf, nrm):
                for c in range(CH):
                    sq = work.tile([P, D], BF16, tag="sq")
                    chain(nc.vector.scalar_tensor_tensor(
                        out=sq, in0=zf[:, c, :], scalar=1.0, in1=zf[:, c, :],
                        op0=ALU.mult, op1=ALU.mult,
                        accum_out=nrm[:, c : c + 1],
                    ))

            def scale_dve(zf, rsq, zs):
                for c in range(CH):
                    chain(nc.vector.tensor_scalar_mul(
                        out=zs[:, c, :], in0=zf[:, c, :],
                        scalar1=rsq[:, c : c + 1],
                    ))

            def xpose_halves(src_bf, dstT):
                """Transpose [P, CH, D] bf16 in 2 halves on both HWDGE queues."""
                flat = src_bf.rearrange("p c d -> p (c d)")
                nc.sync.dma_start_transpose(
                    out=dstT[:, :CH, :], in_=flat[:, : HC * D]
                )
                nc.sync.dma_start_transpose(
                    out=dstT[:, CH:, :], in_=flat[:, HC * D :]
                )

            # ---- DVE: norms-A -> rsq-A -> scale-A (shortest path to zjaT);
            # cast-i after (zi-load-gated anyway); zjaT ahead of ziT in FIFO
            nrm_a = stat.tile([P, CH], F32)
            norms_dve(zja_f, nrm_a)
            rsq_a = stat.tile([P, CH], F32)
            rsqrt_dve(nrm_a, rsq_a, CH)
            zjas = big.tile([P, CH, D], BF16)
            scale_dve(zja_f, rsq_a, zjas)
            zjaT = big.tile([P, CH * KH, P], BF16)
            xpose_halves(zjas, zjaT)
            zjaT_r = zjaT.rearrange("do (c h) m -> do c h m", h=KH)
            zi_bf = big.tile([P, CH, D], BF16)
            chain(nc.vector.tensor_copy(out=zi_bf, in_=zi_f))
            ziT = big.tile([P, CH * KH, P], BF16)
            xpose_halves(zi_bf, ziT)
            ziT_r = ziT.rearrange("do (c h) m -> do c h m", h=KH)

            # ---- zi norms on ScalarE (idle window; square shares exp tables)
            nrm_i = stat.tile([P, CH], F32)
            for c in range(CH):
                sqa = work.tile([P, D], BF16, tag="sqa")
                nc.scalar.activation(
                    out=sqa, in_=zi_f[:, c, :], func=AF.Square,
                    accum_out=nrm_i[:, c : c + 1],
                )
            rsq_i = stat.tile([P, CH], F32)
            rsqrt_dve(nrm_i, rsq_i, CH)
            s2 = stat.tile([P, CH], F32)
            chain(nc.vector.tensor_scalar(
                out=s2, in0=rsq_i, scalar1=2.0, scalar2=None, op0=ALU.mult
            ))
            asch = stat.tile([P, CH], F32)
            chain(nc.vector.tensor_scalar(
                out=asch, in0=s2, scalar1=12102203.161561485, scalar2=None,
                op0=ALU.mult,
            ))
            rsd = stat.tile([P, CH], F32)
            nc.vector.memset(rsd, 0.0)

            # ---- main compute tiles
            rs = psum.tile([P, 2 * CH], F32, tag="rs")  # 0-7: A, 8-15: B
            nc.vector.memset(rs[:, CH:], 0.0)  # DVE-handled B-chunks stay 0

            def half_tile(c, zT_r, g, on_dve=False):
                pt = psum.tile([P, 1024], F32, tag=f"pt{(g * CH + c) % 3}")
                for h in range(KH):
                    for jj in range(2):
                        nc.tensor.matmul(
                            pt[:, jj * 512 : (jj + 1) * 512],
                            lhsT=ziT_r[:, c, h, :],
                            rhs=zT_r[:, jj * 4 : jj * 4 + 4, h, :],
                            start=(h == 0),
                            stop=(h == KH - 1),
                        )
                if not on_dve:
                    nc.scalar.activation(
                        out=pt, in_=pt, func=AF.Exp,
                        scale=s2[:, c : c + 1],
                        accum_out=rs[:, g * CH + c : g * CH + c + 1],
                    )
                else:
                    # Schraudolph exp on DVE: bitcast(i32(A*s2*x + B)) ~ exp;
                    # only used late, when DVE is past all preprocessing
                    ub = work.tile([P, 1024], U32, tag="sch")
                    nc.vector.tensor_scalar(
                        out=ub, in0=pt,
                        scalar1=asch[:, c : c + 1], scalar2=1064986823.0,
                        op0=ALU.mult, op1=ALU.add,
                    )
                    jb = work.tile([P, 1024], BF16, tag="schj")
                    nc.vector.tensor_scalar(
                        out=jb, in0=ub.bitcast(F32),
                        scalar1=1.0, scalar2=0.0, op0=ALU.mult, op1=ALU.add,
                        accum_out=rsd[:, c : c + 1],
                    )

            for c in range(CH):
                half_tile(c, zjaT_r, 0)

            # ---- zjB chain (overlaps A-phase compute)
            nrm_b = stat.tile([P, CH], F32)
            norms_dve(zjb_f, nrm_b)
            rsq_b = stat.tile([P, CH], F32)
            rsqrt_dve(nrm_b, rsq_b, CH)
            zjbs = big.tile([P, CH, D], BF16)
            scale_dve(zjb_f, rsq_b, zjbs)
            zjbT = big.tile([P, CH * KH, P], BF16)
            xpose_halves(zjbs, zjbT)
            zjbT_r = zjbT.rearrange("do (c h) m -> do c h m", h=KH)

            # ---- diag (even cores use it): zi_bf . zjas, then * s2
            dt = stat.tile([P, CH], F32)
            for c in range(CH):
                sqw = work.tile([P, D], BF16, tag="sq")
                chain(nc.vector.scalar_tensor_tensor(
                    out=sqw, in0=zi_bf[:, c, :], scalar=1.0,
                    in1=zjas[:, c, :],
                    op0=ALU.mult, op1=ALU.mult,
                    accum_out=dt[:, c : c + 1],
                ))
            dg = stat.tile([P, CH], F32)
            chain(nc.vector.tensor_mul(out=dg, in0=dt, in1=s2))

            for c in range(CH):
                half_tile(c, zjbT_r, 1, on_dve=(c in (2, 4, 6)))

            # ---- output: [rsA+rsB | dg]; copies float early (deps only),
            # final DMA rides the idle sync queue so Act never issues it
            osb = stat.tile([P, 16], F32)
            nc.vector.tensor_copy(out=osb[:, CH:], in_=dg)
            nc.vector.tensor_copy(out=osb[:, :CH], in_=rs[:, :CH])
            nc.vector.tensor_add(
                out=osb[:, :CH], in0=osb[:, :CH], in1=rs[:, CH:]
            )
            nc.vector.tensor_add(
                out=osb[:, :CH], in0=osb[:, :CH], in1=rsd
            )
            nc.sync.dma_start(out=out, in_=osb)

    nc.compile()
    return nc


_NC = None


def _get_nc():
    global _NC
    if _NC is None:
        _NC = build_nc()
    return _NC


def make_in_maps(z_i, z_j):
    """Per-core inputs for the quarter-block cover."""
    Q = [np.ascontiguousarray(z_j[q * NQ : (q + 1) * NQ], np.float32)
         for q in range(4)]
    in_maps = []
    for q in range(4):
        zi_q = np.ascontiguousarray(z_i[q * NQ : (q + 1) * NQ], np.float32)
        in_maps.append(
            {"z_i": zi_q, "z_ja": Q[q], "z_jb": Q[(q + 1) % 4]}
        )
        in_maps.append(
            {"z_i": zi_q, "z_ja": Q[(q + 2) % 4], "z_jb": Q[(q + 3) % 4]}
        )
    return in_maps


def combine(results):
    """Host: add core-pair exp-sums, ln, subtract diag, mean.

    Row order inside rs/dg is a fixed permutation of the quarter's rows;
    the mean is permutation-invariant and the two cores of a pair share
    the same zi packing, so elementwise add + sum is exact.
    """
    total = 0.0
    for q in range(4):
        oa = results[2 * q]["out"].astype(np.float64)
        ob = results[2 * q + 1]["out"].astype(np.float64)
        rs_total = oa[:, :CH] + ob[:, :CH]   # [128, 8]
        dgq = oa[:, CH:16]                   # diag logits (even core's zjA)
        total += float(np.sum(np.log(rs_total) - dgq))
    return np.float32(total / N)


def kernel(z_i: np.ndarray, z_j: np.ndarray, **_unused) -> np.ndarray:
    z_i = np.ascontiguousarray(z_i, dtype=np.float32)
    z_j = np.ascontiguousarray(z_j, dtype=np.float32)
    nc = _get_nc()
    res = bass_utils.run_bass_kernel_spmd(
        nc, make_in_maps(z_i, z_j), core_ids=list(range(NCORES))
    )
    return combine(res.results)


# revision 43
# speedup vs baseline: 1.0550x; 1.0083x over previous
"""Contrastive loss (SimCLR-style) on 8 TRN2 NeuronCores.

loss = -mean(diag(log_softmax(zi_n @ zj_n^T / T)))  with zi_n, zj_n L2-normalized,
N=4096, D=256, T=0.5.

Sharding: 16 quarter-block cover. Core pair (2q, 2q+1) owns zi quarter q
(1024 rows). Core 2q gets zj cols = quarters {q, q+1}; core 2q+1 gets
{q+2, q+3} (mod 4). Every (row-quarter, col-quarter) block is computed
exactly once; every core's zjA input aligns row-for-row with its zi input,
so the diagonal block is always in zjA for even cores. 3MB HBM per core.

Per core: l2-normalize zj (bf16, quake rsqrt on DVE), zi norms on the
otherwise-idle ScalarE (Square+accum shares the exp table set), xbar
transposes split across both HWDGE queues, bf16 matmul with PE pre-warm
against the HAM clock gate, fused exp+row-sum on ScalarE with
per-partition scale 2/||zi||. Device outputs per-row partial exp-sums +
diag logits; host adds core-pair partials, takes ln, subtracts diag, means.
"""

import numpy as np

import concourse.bass as bass
import concourse.bacc as bacc
import concourse.tile as tile
import concourse.bass_utils as bass_utils
from concourse import mybir
from concourse.tile_rust import add_dep_helper

N = 4096
D = 256
NCORES = 8
NQ = N // 4          # 1024 rows per quarter (zi rows per core)
P = 128
CH = NQ // P         # 8 row-chunks (rows per partition)
HC = CH // 2
KH = D // P          # 2 contraction halves
MAGIC = 0x5F3759DF
NWARM = 16           # dummy matmuls to lift the PE HAM clock gate

F32 = mybir.dt.float32
U32 = mybir.dt.uint32
BF16 = mybir.dt.bfloat16
AF = mybir.ActivationFunctionType
ALU = mybir.AluOpType
AX = mybir.AxisListType


def build_nc():
    nc = bacc.Bacc(
        "TRN2",
        target_bir_lowering=False,
        debug=False,
        enable_asserts=False,
    )
    z_i = nc.dram_tensor("z_i", (NQ, D), F32, kind="ExternalInput").ap()
    z_ja = nc.dram_tensor("z_ja", (NQ, D), F32, kind="ExternalInput").ap()
    z_jb = nc.dram_tensor("z_jb", (NQ, D), F32, kind="ExternalInput").ap()
    out = nc.dram_tensor("out", (P, 16), F32, kind="ExternalOutput").ap()

    with tile.TileContext(nc) as tc:
        with (
            tc.tile_pool(name="const", bufs=1) as const,
            tc.tile_pool(name="big", bufs=1) as big,
            tc.tile_pool(name="work", bufs=2) as work,
            tc.tile_pool(name="stat", bufs=1) as stat,
            tc.tile_pool(name="psum", bufs=1, space="PSUM") as psum,
            tc.tile_pool(name="warm", bufs=1, space="PSUM") as warmp,
        ):
            # pin the exp ACT table set at t=0 (also covers square/copy)
            dummy = const.tile([1, 1], F32)
            nc.vector.memset(dummy, 1.0)
            nc.scalar.activation(out=dummy, in_=dummy, func=AF.Exp)

            magic = const.tile([P, 16], U32)
            nc.vector.memset(magic, MAGIC)

            # ---- loads (sync queue, FIFO): zjA halves -> zi halves -> zjB
            def load_halves(dst, src):
                for hh in range(2):
                    nc.sync.dma_start(
                        out=dst[:, hh * HC : (hh + 1) * HC, :],
                        in_=src[hh * 512 : (hh + 1) * 512, :].rearrange(
                            "(p c) d -> p c d", p=P
                        ),
                    )

            zja_f = big.tile([P, CH, D], F32)
            load_halves(zja_f, z_ja)
            zi_f = big.tile([P, CH, D], F32)
            load_halves(zi_f, z_i)
            zjb_f = big.tile([P, CH, D], F32)
            nc.sync.dma_start(
                out=zjb_f, in_=z_jb.rearrange("(p c) d -> p c d", p=P)
            )

            # ---- PE warm-up against the HAM clock gate
            wsrc = const.tile([P, 512], BF16)
            nc.vector.memset(wsrc, 0.0)
            wp = warmp.tile([P, 512], F32, tag="wp")
            for _ in range(NWARM):
                nc.tensor.matmul(
                    wp, lhsT=wsrc[:, :P], rhs=wsrc, start=True, stop=True,
                    skip_group_check=True,
                )

            dve_tail = [None]

            def chain(bi):
                if dve_tail[0] is not None:
                    add_dep_helper(bi.ins, dve_tail[0], sync=False,
                                   reason="dve order")
                dve_tail[0] = bi.ins
                return bi

            def rsqrt_dve(a, y, w):
                """y[:,:w] = 1/sqrt(a[:,:w]): quake seed + 1 Newton step."""
                au = a.bitcast(U32)
                yu = y.bitcast(U32)
                sh = work.tile([P, 16], U32, tag="rsq_sh")
                chain(nc.vector.tensor_scalar(
                    out=sh[:, :w], in0=au, scalar1=1, scalar2=None,
                    op0=ALU.logical_shift_right,
                ))
                chain(nc.vector.tensor_sub(
                    out=yu, in0=magic[:, :w], in1=sh[:, :w]
                ))
                t1 = work.tile([P, 16], F32, tag="rsq_t1")
                chain(nc.vector.tensor_mul(out=t1[:, :w], in0=y, in1=y))
                chain(nc.vector.tensor_mul(out=t1[:, :w], in0=t1[:, :w], in1=a))
                chain(nc.vector.tensor_scalar(
                    out=t1[:, :w], in0=t1[:, :w], scalar1=-0.5, scalar2=1.5,
                    op0=ALU.mult, op1=ALU.add,
                ))
                chain(nc.vector.tensor_mul(out=y, in0=y, in1=t1[:, :w]))

            def norms_dve(zf, nrm):
                for c in range(CH):
                    sq = work.tile([P, D], BF16, tag="sq")
                    chain(nc.vector.scalar_tensor_tensor(
                        out=sq, in0=zf[:, c, :], scalar=1.0, in1=zf[:, c, :],
                        op0=ALU.mult, op1=ALU.mult,
                        accum_out=nrm[:, c : c + 1],
                    ))

            def scale_dve(zf, rsq, zs):
                for c in range(CH):
                    chain(nc.vector.tensor_scalar_mul(
                        out=zs[:, c, :], in0=zf[:, c, :],
                        scalar1=rsq[:, c : c + 1],
                    ))

            def xpose_halves(src_bf, dstT):
                """Transpose [P, CH, D] bf16 in 2 halves on both HWDGE queues."""
                flat = src_bf.rearrange("p c d -> p (c d)")
                nc.sync.dma_start_transpose(
                    out=dstT[:, :CH, :], in_=flat[:, : HC * D]
                )
                nc.sync.dma_start_transpose(
                    out=dstT[:, CH:, :], in_=flat[:, HC * D :]
                )

            # ---- DVE: norms-A -> rsq-A -> scale-A (shortest path to zjaT);
            # cast-i after (zi-load-gated anyway); zjaT ahead of ziT in FIFO
            nrm_a = stat.tile([P, CH], F32)
            norms_dve(zja_f, nrm_a)
            rsq_a = stat.tile([P, CH], F32)
            rsqrt_dve(nrm_a, rsq_a, CH)
            zjas = big.tile([P, CH, D], BF16)
            scale_dve(zja_f, rsq_a, zjas)
            zjaT = big.tile([P, CH * KH, P], BF16)
            xpose_halves(zjas, zjaT)
            zjaT_r = zjaT.rearrange("do (c h) m -> do c h m", h=KH)
            zi_bf = big.tile([P, CH, D], BF16)
            chain(nc.vector.tensor_copy(out=zi_bf, in_=zi_f))
            ziT = big.tile([P, CH * KH, P], BF16)
            xpose_halves(zi_bf, ziT)
            ziT_r = ziT.rearrange("do (c h) m -> do c h m", h=KH)

            # ---- zi norms on ScalarE (idle window; square shares exp tables)
            nrm_i = stat.tile([P, CH], F32)
            for c in range(CH):
                sqa = work.tile([P, D], BF16, tag="sqa")
                nc.scalar.activation(
                    out=sqa, in_=zi_f[:, c, :], func=AF.Square,
                    accum_out=nrm_i[:, c : c + 1],
                )
            rsq_i = stat.tile([P, CH], F32)
            rsqrt_dve(nrm_i, rsq_i, CH)
            s2 = stat.tile([P, CH], F32)
            chain(nc.vector.tensor_scalar(
                out=s2, in0=rsq_i, scalar1=2.0, scalar2=None, op0=ALU.mult
            ))
            asch = stat.tile([P, CH], F32)
            chain(nc.vector.tensor_scalar(
                out=asch, in0=s2, scalar1=12102203.161561485, scalar2=None,
                op0=ALU.mult,
            ))
            rsd = stat.tile([P, CH], F32)
            nc.vector.memset(rsd, 0.0)

            # ---- main compute tiles
            rs = psum.tile([P, 2 * CH], F32, tag="rs")  # 0-7: A, 8-15: B
            nc.vector.memset(rs[:, 7:], 0.0)  # DVE-handled chunk cols stay 0

            def dve_exp(pt, c, acc):
                ub = work.tile([P, 1024], U32, tag="sch")
                nc.vector.tensor_scalar(
                    out=ub, in0=pt,
                    scalar1=asch[:, c : c + 1], scalar2=1064986823.0,
                    op0=ALU.mult, op1=ALU.add,
                )
                jb = work.tile([P, 1024], BF16, tag="schj")
                nc.vector.tensor_scalar(
                    out=jb, in0=ub.bitcast(F32),
                    scalar1=1.0, scalar2=0.0, op0=ALU.mult, op1=ALU.add,
                    accum_out=acc,
                )

            def half_tile(c, zT_r, g, on_dve=False, defer=False):
                pt = psum.tile([P, 1024], F32, tag=f"pt{(g * CH + c) % 3}")
                for h in range(KH):
                    for jj in range(2):
                        nc.tensor.matmul(
                            pt[:, jj * 512 : (jj + 1) * 512],
                            lhsT=ziT_r[:, c, h, :],
                            rhs=zT_r[:, jj * 4 : jj * 4 + 4, h, :],
                            start=(h == 0),
                            stop=(h == KH - 1),
                        )
                if defer:
                    return pt
                if not on_dve:
                    nc.scalar.activation(
                        out=pt, in_=pt, func=AF.Exp,
                        scale=s2[:, c : c + 1],
                        accum_out=rs[:, g * CH + c : g * CH + c + 1],
                    )
                else:
                    dve_exp(pt, c, rsd[:, c : c + 1])

            pt_a7 = None
            for c in range(CH):
                r = half_tile(c, zjaT_r, 0, defer=(c == 7))
                if c == 7:
                    pt_a7 = r

            # ---- zjB chain (overlaps A-phase compute)
            nrm_b = stat.tile([P, CH], F32)
            norms_dve(zjb_f, nrm_b)
            rsq_b = stat.tile([P, CH], F32)
            rsqrt_dve(nrm_b, rsq_b, CH)
            zjbs = big.tile([P, CH, D], BF16)
            scale_dve(zjb_f, rsq_b, zjbs)
            zjbT = big.tile([P, CH * KH, P], BF16)
            xpose_halves(zjbs, zjbT)
            zjbT_r = zjbT.rearrange("do (c h) m -> do c h m", h=KH)

            # ---- diag (even cores use it): zi_bf . zjas, then * s2
            dt = stat.tile([P, CH], F32)
            for c in range(CH):
                sqw = work.tile([P, D], BF16, tag="sq")
                chain(nc.vector.scalar_tensor_tensor(
                    out=sqw, in0=zi_bf[:, c, :], scalar=1.0,
                    in1=zjas[:, c, :],
                    op0=ALU.mult, op1=ALU.mult,
                    accum_out=dt[:, c : c + 1],
                ))
            dg = stat.tile([P, CH], F32)
            chain(nc.vector.tensor_mul(out=dg, in0=dt, in1=s2))

            dve_exp(pt_a7, 7, rsd[:, 7:8])
            for c in range(CH):
                half_tile(c, zjbT_r, 1, on_dve=(c in (2, 4, 6)))

            # ---- output: [rsA+rsB | dg]; copies float early (deps only),
            # final DMA rides the idle sync queue so Act never issues it
            osb = stat.tile([P, 16], F32)
            nc.vector.tensor_copy(out=osb[:, CH:], in_=dg)
            nc.vector.tensor_copy(out=osb[:, :CH], in_=rs[:, :CH])
            nc.vector.tensor_add(
                out=osb[:, :CH], in0=osb[:, :CH], in1=rs[:, CH:]
            )
            nc.vector.tensor_add(
                out=osb[:, :CH], in0=osb[:, :CH], in1=rsd
            )
            nc.sync.dma_start(out=out, in_=osb)

    nc.compile()
    return nc


_NC = None


def _get_nc():
    global _NC
    if _NC is None:
        _NC = build_nc()
    return _NC


def make_in_maps(z_i, z_j):
    """Per-core inputs for the quarter-block cover."""
    Q = [np.ascontiguousarray(z_j[q * NQ : (q + 1) * NQ], np.float32)
         for q in range(4)]
    in_maps = []
    for q in range(4):
        zi_q = np.ascontiguousarray(z_i[q * NQ : (q + 1) * NQ], np.float32)
        in_maps.append(
            {"z_i": zi_q, "z_ja": Q[q], "z_jb": Q[(q + 1) % 4]}
        )
        in_maps.append(
            {"z_i": zi_q, "z_ja": Q[(q + 2) % 4], "z_jb": Q[(q + 3) % 4]}
        )
    return in_maps


def combine(results):
    """Host: add core-pair exp-sums, ln, subtract diag, mean.

    Row order inside rs/dg is a fixed permutation of the quarter's rows;
    the mean is permutation-invariant and the two cores of a pair share
    the same zi packing, so elementwise add + sum is exact.
    """
    total = 0.0
    for q in range(4):
        oa = results[2 * q]["out"].astype(np.float64)
        ob = results[2 * q + 1]["out"].astype(np.float64)
        rs_total = oa[:, :CH] + ob[:, :CH]   # [128, 8]
        dgq = oa[:, CH:16]                   # diag logits (even core's zjA)
        total += float(np.sum(np.log(rs_total) - dgq))
    return np.float32(total / N)


def kernel(z_i: np.ndarray, z_j: np.ndarray, **_unused) -> np.ndarray:
    z_i = np.ascontiguousarray(z_i, dtype=np.float32)
    z_j = np.ascontiguousarray(z_j, dtype=np.float32)
    nc = _get_nc()
    res = bass_utils.run_bass_kernel_spmd(
        nc, make_in_maps(z_i, z_j), core_ids=list(range(NCORES))
    )
    return combine(res.results)


# revision 44
# speedup vs baseline: 1.0767x; 1.0206x over previous
"""Contrastive loss (SimCLR-style) on 8 TRN2 NeuronCores.

loss = -mean(diag(log_softmax(zi_n @ zj_n^T / T)))  with zi_n, zj_n L2-normalized,
N=4096, D=256, T=0.5.

Sharding: 16 quarter-block cover. Core pair (2q, 2q+1) owns zi quarter q
(1024 rows). Core 2q gets zj cols = quarters {q, q+1}; core 2q+1 gets
{q+2, q+3} (mod 4). Every (row-quarter, col-quarter) block is computed
exactly once; every core's zjA input aligns row-for-row with its zi input,
so the diagonal block is always in zjA for even cores. 3MB HBM per core.

Per core: l2-normalize zj (bf16, quake rsqrt on DVE), zi norms on the
otherwise-idle ScalarE (Square+accum shares the exp table set), xbar
transposes split across both HWDGE queues, bf16 matmul with PE pre-warm
against the HAM clock gate, fused exp+row-sum on ScalarE with
per-partition scale 2/||zi||. Device outputs per-row partial exp-sums +
diag logits; host adds core-pair partials, takes ln, subtracts diag, means.
"""

import numpy as np

import concourse.bass as bass
import concourse.bacc as bacc
import concourse.tile as tile
import concourse.bass_utils as bass_utils
from concourse import mybir
from concourse.tile_rust import add_dep_helper

N = 4096
D = 256
NCORES = 8
NQ = N // 4          # 1024 rows per quarter (zi rows per core)
P = 128
CH = NQ // P         # 8 row-chunks (rows per partition)
HC = CH // 2
KH = D // P          # 2 contraction halves
MAGIC = 0x5F3759DF
NWARM = 16           # dummy matmuls to lift the PE HAM clock gate

F32 = mybir.dt.float32
U32 = mybir.dt.uint32
BF16 = mybir.dt.bfloat16
AF = mybir.ActivationFunctionType
ALU = mybir.AluOpType
AX = mybir.AxisListType


def build_nc():
    nc = bacc.Bacc(
        "TRN2",
        target_bir_lowering=False,
        debug=False,
        enable_asserts=False,
    )
    z_i = nc.dram_tensor("z_i", (NQ, D), F32, kind="ExternalInput").ap()
    z_ja = nc.dram_tensor("z_ja", (NQ, D), F32, kind="ExternalInput").ap()
    z_jb = nc.dram_tensor("z_jb", (NQ, D), F32, kind="ExternalInput").ap()
    out = nc.dram_tensor("out", (P, 16), F32, kind="ExternalOutput").ap()

    with tile.TileContext(nc) as tc:
        with (
            tc.tile_pool(name="const", bufs=1) as const,
            tc.tile_pool(name="big", bufs=1) as big,
            tc.tile_pool(name="work", bufs=2) as work,
            tc.tile_pool(name="stat", bufs=1) as stat,
            tc.tile_pool(name="psum", bufs=1, space="PSUM") as psum,
            tc.tile_pool(name="warm", bufs=1, space="PSUM") as warmp,
        ):
            # pin the exp ACT table set at t=0 (also covers square/copy)
            dummy = const.tile([1, 1], F32)
            nc.vector.memset(dummy, 1.0)
            nc.scalar.activation(out=dummy, in_=dummy, func=AF.Exp)

            magic = const.tile([P, 16], U32)
            nc.vector.memset(magic, MAGIC)

            # ---- loads (sync queue, FIFO): zjA halves -> zi halves -> zjB
            def load_halves(dst, src):
                for hh in range(2):
                    nc.sync.dma_start(
                        out=dst[:, hh * HC : (hh + 1) * HC, :],
                        in_=src[hh * 512 : (hh + 1) * 512, :].rearrange(
                            "(p c) d -> p c d", p=P
                        ),
                    )

            zja_f = big.tile([P, CH, D], F32)
            load_halves(zja_f, z_ja)
            zi_f = big.tile([P, CH, D], F32)
            load_halves(zi_f, z_i)
            zjb_f = big.tile([P, CH, D], F32)
            nc.sync.dma_start(
                out=zjb_f, in_=z_jb.rearrange("(p c) d -> p c d", p=P)
            )

            # ---- PE warm-up against the HAM clock gate
            wsrc = const.tile([P, 512], BF16)
            nc.vector.memset(wsrc, 0.0)
            wp = warmp.tile([P, 512], F32, tag="wp")
            for _ in range(NWARM):
                nc.tensor.matmul(
                    wp, lhsT=wsrc[:, :P], rhs=wsrc, start=True, stop=True,
                    skip_group_check=True,
                )

            dve_tail = [None]

            def chain(bi):
                if dve_tail[0] is not None:
                    add_dep_helper(bi.ins, dve_tail[0], sync=False,
                                   reason="dve order")
                dve_tail[0] = bi.ins
                return bi

            def rsqrt_dve(a, y, w):
                """y[:,:w] = 1/sqrt(a[:,:w]): quake seed + 1 Newton step."""
                au = a.bitcast(U32)
                yu = y.bitcast(U32)
                sh = work.tile([P, 16], U32, tag="rsq_sh")
                chain(nc.vector.tensor_scalar(
                    out=sh[:, :w], in0=au, scalar1=1, scalar2=None,
                    op0=ALU.logical_shift_right,
                ))
                chain(nc.vector.tensor_sub(
                    out=yu, in0=magic[:, :w], in1=sh[:, :w]
                ))
                t1 = work.tile([P, 16], F32, tag="rsq_t1")
                chain(nc.vector.tensor_mul(out=t1[:, :w], in0=y, in1=y))
                chain(nc.vector.tensor_mul(out=t1[:, :w], in0=t1[:, :w], in1=a))
                chain(nc.vector.tensor_scalar(
                    out=t1[:, :w], in0=t1[:, :w], scalar1=-0.5, scalar2=1.5,
                    op0=ALU.mult, op1=ALU.add,
                ))
                chain(nc.vector.tensor_mul(out=y, in0=y, in1=t1[:, :w]))

            def norms_dve(zf, nrm):
                for c in range(CH):
                    sq = work.tile([P, D], BF16, tag="sq")
                    chain(nc.vector.scalar_tensor_tensor(
                        out=sq, in0=zf[:, c, :], scalar=1.0, in1=zf[:, c, :],
                        op0=ALU.mult, op1=ALU.mult,
                        accum_out=nrm[:, c : c + 1],
                    ))

            def scale_dve(zf, rsq, zs):
                for c in range(CH):
                    chain(nc.vector.tensor_scalar_mul(
                        out=zs[:, c, :], in0=zf[:, c, :],
                        scalar1=rsq[:, c : c + 1],
                    ))

            def xpose_halves(src_bf, dstT):
                """Transpose [P, CH, D] bf16 in 2 halves on both HWDGE queues."""
                flat = src_bf.rearrange("p c d -> p (c d)")
                nc.sync.dma_start_transpose(
                    out=dstT[:, :CH, :], in_=flat[:, : HC * D]
                )
                nc.sync.dma_start_transpose(
                    out=dstT[:, CH:, :], in_=flat[:, HC * D :]
                )

            # ---- DVE: norms-A -> rsq-A -> scale-A (shortest path to zjaT);
            # cast-i after (zi-load-gated anyway); zjaT ahead of ziT in FIFO
            nrm_a = stat.tile([P, CH], F32)
            norms_dve(zja_f, nrm_a)
            rsq_a = stat.tile([P, CH], F32)
            rsqrt_dve(nrm_a, rsq_a, CH)
            zjas = big.tile([P, CH, D], BF16)
            scale_dve(zja_f, rsq_a, zjas)
            zjaT = big.tile([P, CH * KH, P], BF16)
            xpose_halves(zjas, zjaT)
            zjaT_r = zjaT.rearrange("do (c h) m -> do c h m", h=KH)
            zi_bf = big.tile([P, CH, D], BF16)
            chain(nc.vector.tensor_copy(out=zi_bf, in_=zi_f))
            ziT = big.tile([P, CH * KH, P], BF16)
            xpose_halves(zi_bf, ziT)
            ziT_r = ziT.rearrange("do (c h) m -> do c h m", h=KH)

            # ---- zi norms on ScalarE (idle window; square shares exp tables)
            nrm_i = stat.tile([P, CH], F32)
            for c in range(CH):
                sqa = work.tile([P, D], BF16, tag="sqa")
                nc.scalar.activation(
                    out=sqa, in_=zi_f[:, c, :], func=AF.Square,
                    accum_out=nrm_i[:, c : c + 1],
                )
            rsq_i = stat.tile([P, CH], F32)
            rsqrt_dve(nrm_i, rsq_i, CH)
            s2 = stat.tile([P, CH], F32)
            chain(nc.vector.tensor_scalar(
                out=s2, in0=rsq_i, scalar1=2.0, scalar2=None, op0=ALU.mult
            ))
            asch = stat.tile([P, CH], F32)
            chain(nc.vector.tensor_scalar(
                out=asch, in0=s2, scalar1=12102203.161561485, scalar2=None,
                op0=ALU.mult,
            ))
            rsd = stat.tile([P, CH], F32)
            nc.vector.memset(rsd, 0.0)

            # ---- main compute tiles
            rs = psum.tile([P, 2 * CH], F32, tag="rs")  # 0-7: A, 8-15: B
            nc.vector.memset(rs[:, 6:], 0.0)  # DVE-handled chunk cols stay 0

            def dve_exp(pt, c, acc):
                ub = work.tile([P, 1024], U32, tag="sch")
                nc.vector.tensor_scalar(
                    out=ub, in0=pt,
                    scalar1=asch[:, c : c + 1], scalar2=1064986823.0,
                    op0=ALU.mult, op1=ALU.add,
                )
                jb = work.tile([P, 1024], BF16, tag="schj")
                nc.vector.tensor_scalar(
                    out=jb, in0=ub.bitcast(F32),
                    scalar1=1.0, scalar2=0.0, op0=ALU.mult, op1=ALU.add,
                    accum_out=acc,
                )

            def half_tile(c, zT_r, g, on_dve=False, defer=False):
                pt = psum.tile([P, 1024], F32, tag=f"pt{(g * CH + c) % 3}")
                for h in range(KH):
                    for jj in range(2):
                        nc.tensor.matmul(
                            pt[:, jj * 512 : (jj + 1) * 512],
                            lhsT=ziT_r[:, c, h, :],
                            rhs=zT_r[:, jj * 4 : jj * 4 + 4, h, :],
                            start=(h == 0),
                            stop=(h == KH - 1),
                        )
                if defer:
                    return pt
                if not on_dve:
                    nc.scalar.activation(
                        out=pt, in_=pt, func=AF.Exp,
                        scale=s2[:, c : c + 1],
                        accum_out=rs[:, g * CH + c : g * CH + c + 1],
                    )
                else:
                    dve_exp(pt, c, rsd[:, c : c + 1])

            pt_a = {}
            for c in range(CH):
                r = half_tile(c, zjaT_r, 0, defer=(c >= 6))
                if c >= 6:
                    pt_a[c] = r

            # ---- zjB chain (overlaps A-phase compute)
            nrm_b = stat.tile([P, CH], F32)
            norms_dve(zjb_f, nrm_b)
            rsq_b = stat.tile([P, CH], F32)
            rsqrt_dve(nrm_b, rsq_b, CH)
            zjbs = big.tile([P, CH, D], BF16)
            scale_dve(zjb_f, rsq_b, zjbs)
            zjbT = big.tile([P, CH * KH, P], BF16)
            xpose_halves(zjbs, zjbT)
            zjbT_r = zjbT.rearrange("do (c h) m -> do c h m", h=KH)

            # ---- diag (even cores use it): zi_bf . zjas, then * s2
            dt = stat.tile([P, CH], F32)
            for c in range(CH):
                sqw = work.tile([P, D], BF16, tag="sq")
                chain(nc.vector.scalar_tensor_tensor(
                    out=sqw, in0=zi_bf[:, c, :], scalar=1.0,
                    in1=zjas[:, c, :],
                    op0=ALU.mult, op1=ALU.mult,
                    accum_out=dt[:, c : c + 1],
                ))
            dg = stat.tile([P, CH], F32)
            chain(nc.vector.tensor_mul(out=dg, in0=dt, in1=s2))

            dve_exp(pt_a[6], 6, rsd[:, 6:7])
            dve_exp(pt_a[7], 7, rsd[:, 7:8])
            for c in range(CH):
                half_tile(c, zjbT_r, 1, on_dve=(c in (2, 4, 6)))

            # ---- output: [rsA+rsB | dg]; copies float early (deps only),
            # final DMA rides the idle sync queue so Act never issues it
            osb = stat.tile([P, 16], F32)
            nc.vector.tensor_copy(out=osb[:, CH:], in_=dg)
            nc.vector.tensor_copy(out=osb[:, :CH], in_=rs[:, :CH])
            nc.vector.tensor_add(
                out=osb[:, :CH], in0=osb[:, :CH], in1=rs[:, CH:]
            )
            nc.vector.tensor_add(
                out=osb[:, :CH], in0=osb[:, :CH], in1=rsd
            )
            nc.sync.dma_start(out=out, in_=osb)

    nc.compile()
    return nc


_NC = None


def _get_nc():
    global _NC
    if _NC is None:
        _NC = build_nc()
    return _NC


def make_in_maps(z_i, z_j):
    """Per-core inputs for the quarter-block cover."""
    Q = [np.ascontiguousarray(z_j[q * NQ : (q + 1) * NQ], np.float32)
         for q in range(4)]
    in_maps = []
    for q in range(4):
        zi_q = np.ascontiguousarray(z_i[q * NQ : (q + 1) * NQ], np.float32)
        in_maps.append(
            {"z_i": zi_q, "z_ja": Q[q], "z_jb": Q[(q + 1) % 4]}
        )
        in_maps.append(
            {"z_i": zi_q, "z_ja": Q[(q + 2) % 4], "z_jb": Q[(q + 3) % 4]}
        )
    return in_maps


def combine(results):
    """Host: add core-pair exp-sums, ln, subtract diag, mean.

    Row order inside rs/dg is a fixed permutation of the quarter's rows;
    the mean is permutation-invariant and the two cores of a pair share
    the same zi packing, so elementwise add + sum is exact.
    """
    total = 0.0
    for q in range(4):
        oa = results[2 * q]["out"].astype(np.float64)
        ob = results[2 * q + 1]["out"].astype(np.float64)
        rs_total = oa[:, :CH] + ob[:, :CH]   # [128, 8]
        dgq = oa[:, CH:16]                   # diag logits (even core's zjA)
        total += float(np.sum(np.log(rs_total) - dgq))
    return np.float32(total / N)


def kernel(z_i: np.ndarray, z_j: np.ndarray, **_unused) -> np.ndarray:
    z_i = np.ascontiguousarray(z_i, dtype=np.float32)
    z_j = np.ascontiguousarray(z_j, dtype=np.float32)
    nc = _get_nc()
    res = bass_utils.run_bass_kernel_spmd(
        nc, make_in_maps(z_i, z_j), core_ids=list(range(NCORES))
    )
    return combine(res.results)
